# revision 7
# baseline (speedup 1.0000x reference)
"""DDiT block (AdaLN-modulated transformer block) on 8 Trainium2 NeuronCores.

Sharding: pure data-parallel, core = (batch b in {0,1}) x (query-chunk k in
0..3 of 512 tokens).  Each core computes LN1/K/V over the full 2048-token
batch (K/V replicated within the 4 cores of a batch — avoids any collective),
then attention / out-proj / LN2 / MLP for its own 512 queries.  AdaLN
modulation vectors are computed host-side and replicated.

Activations are kept transposed ([d on partitions, t on free]):
  - LN stats (reduce over d) are ones-vector matmuls on the PE (the ones
    column is pre-scaled by 1/D so the PSUM rows are mean / E[x^2] directly),
  - per-token rows (LN mu/rstd, softmax 1/denom) are broadcast across
    partitions with a [1,128]-stationary ones matmul into PSUM (no DRAM
    bounce DMA),
  - the softmax denominator comes from a ones-column appended to V (AV
    matmul row 64 = sum of exp).

x is loaded in bf16 (halves the 6.3MB startup read); all big GEMMs run in
bf16; LN statistics, softmax denominators and the residual accumulator stay
in fp32.

Engine balance: PE does all GEMMs + broadcasts; Act does LN modulate /
softmax exp / gelu; DVE does LN rows, PSUM evacuation and reciprocals;
the otherwise-idle Pool (gpsimd) engine does SBUF-only squares / subtracts /
residual adds (Pool cannot touch PSUM).

Schedule: V-projection and Q are interleaved into the first two heads'
score/exp units; AV(h) is interleaved into SC(h+2)'s units so the Act
engine (softmax exp, the attention bottleneck) never waits; LN2 statistics
matmuls are interleaved into the out-projection accumulation groups.
"""

import numpy as np

import concourse.bass as bass
import concourse.mybir as mybir
import concourse.tile as tile
from concourse.bass_utils import run_bass_kernel_spmd

F32 = mybir.dt.float32
F32R = mybir.dt.float32r
BF16 = mybir.dt.bfloat16
AF = mybir.ActivationFunctionType
OP = mybir.AluOpType

D = 768
S = 2048
H = 12
DH = 64
DC = D // 128          # 6 chunks of d on partitions
HID = 4 * D            # 3072
HC = HID // 128        # 24
NQ = 512               # queries per core
NCH = S // NQ          # 4 token chunks
NTP = S // 128         # 16 key chunks of 128
EPS = 1e-5


def _body(tc, dram):
    nc = tc.nc
    r128 = lambda name: dram[name].ap().rearrange("(o p) j -> p o j", p=128)
    xT_r = r128("xT")
    wqkv_r = r128("w_qkvT")
    wout_r = r128("w_outT")
    wm1_r = r128("w_m1T")
    wm2_r = r128("w_m2T")
    outT_r = r128("outT")

    import contextlib
    with contextlib.ExitStack() as ctx:
        main = ctx.enter_context(tc.tile_pool(name="main", bufs=1))
        psmm = ctx.enter_context(tc.tile_pool(name="psmm", bufs=2, space="PSUM"))
        sqp = ctx.enter_context(tc.tile_pool(name="sqp", bufs=1))
        rowsp = ctx.enter_context(tc.tile_pool(name="rows", bufs=2))
        wqp = ctx.enter_context(tc.tile_pool(name="wq", bufs=3))
        xmp = ctx.enter_context(tc.tile_pool(name="xmp", bufs=1))
        kqp = ctx.enter_context(tc.tile_pool(name="kqp", bufs=1))
        vp = ctx.enter_context(tc.tile_pool(name="vp", bufs=1))
        wmp = ctx.enter_context(tc.tile_pool(name="wmp", bufs=2))

        # ---- constants (gpsimd DMA queue; sync queue is reserved for x/w)
        ones_f32 = main.tile([128, 1], F32)
        nc.vector.memset(ones_f32, 1.0)
        ones_row = main.tile([1, 128], F32R)          # bcast stationary
        nc.gpsimd.dma_start(ones_row, dram["ones_row"].ap())
        onesb_col = main.tile([128, 1], BF16)         # 1/D, LN stats (bf16)
        nc.gpsimd.dma_start(onesb_col, dram["onesb_col"].ap())
        onesf_col = main.tile([128, 1], F32R)         # 1/D, LN stats (f32r)
        nc.gpsimd.dma_start(onesf_col, dram["onesf_col"].ap())
        ada = main.tile([128, 36], F32)
        nc.gpsimd.dma_start(ada, dram["ada_c"].ap())
        n1_sb = main.tile([128, DC], F32)
        nc.gpsimd.dma_start(n1_sb, dram["n1_c"].ap())
        n2_sb = main.tile([128, DC], F32)
        nc.gpsimd.dma_start(n2_sb, dram["n2_c"].ap())
        b1_sb = main.tile([128, HC], F32)
        nc.gpsimd.dma_start(b1_sb, dram["b1_c"].ap())
        b2_sb = main.tile([128, DC], F32)
        nc.gpsimd.dma_start(b2_sb, dram["b2_c"].ap())

        sh_msa, sc_msa, g_msa = ada[:, 0:6], ada[:, 6:12], ada[:, 12:18]
        sh_mlp, sc_mlp, g_mlp = ada[:, 18:24], ada[:, 24:30], ada[:, 30:36]
        a1 = main.tile([128, DC], F32)
        nc.vector.scalar_tensor_tensor(a1, in0=sc_msa, scalar=1.0, in1=n1_sb,
                                       op0=OP.add, op1=OP.mult)
        a2 = main.tile([128, DC], F32)
        nc.vector.scalar_tensor_tensor(a2, in0=sc_mlp, scalar=1.0, in1=n2_sb,
                                       op0=OP.add, op1=OP.mult)
        gb2 = main.tile([128, DC], F32)
        nc.vector.tensor_mul(gb2, g_mlp, b2_sb)

        oT = main.tile([128, DC, NQ], BF16)
        xskip = main.tile([128, DC, NQ], BF16)
        x2 = main.tile([128, DC, NQ], F32R)

        # ---- x chunks (bf16) + first QKV weight halves, on the sync queue.
        # Chunk 0 is dispatched first so LN(0) starts ASAP.
        xp_ctx = tc.tile_pool(name="xp", bufs=1)
        xp = xp_ctx.__enter__()
        psln_ctx = tc.tile_pool(name="psln", bufs=2, space="PSUM")
        psln = psln_ctx.__enter__()
        psbc_ctx = tc.tile_pool(name="psbc", bufs=2, space="PSUM")
        psbc = psbc_ctx.__enter__()

        x_t = []
        for ch in range(NCH):
            xt = xp.tile([128, DC, NQ], BF16, name=f"x{ch}")
            nc.sync.dma_start(xt, xT_r[:, :, ch * NQ:(ch + 1) * NQ])
            x_t.append(xt)
            if ch == 0:
                wk = []
                for half in range(2):
                    w = wqp.tile([128, DC, 384], BF16, tag="w")
                    nc.sync.dma_start(
                        w, wqkv_r[:, :, D + half * 384:D + (half + 1) * 384])
                    wk.append(w)

        def modulate(xsrc, pslnp, psbcp, a_col, sh_col, xm, ones_col, sq_mm):
            """LN + AdaLN modulate, one [128, DC, NQ] chunk -> xm (bf16).
            sq_mm(s) emits the s2 stat matmuls (so LN1 can use prepared bf16
            squares while LN2 squares f32r x2 on the fly)."""
            s = pslnp.tile([65, NQ], F32, tag="s")
            for o in range(DC):
                nc.tensor.matmul(s[0:1, :], ones_col, xsrc[:, o, :],
                                 start=(o == 0), stop=(o == DC - 1))
            sq_mm(s)
            mu_sb = rowsp.tile([1, NQ], F32R, tag="mu")
            nc.vector.tensor_copy(mu_sb, s[0:1, :])
            musq = rowsp.tile([1, NQ], F32, tag="msq", bufs=1)
            nc.gpsimd.tensor_mul(musq, mu_sb, mu_sb)
            var = rowsp.tile([1, NQ], F32, tag="var", bufs=1)
            nc.vector.scalar_tensor_tensor(var, in0=s[64:65, :], scalar=EPS,
                                           in1=musq, op0=OP.add,
                                           op1=OP.subtract)
            rvar = rowsp.tile([1, NQ], F32, tag="rv", bufs=1)
            nc.vector.reciprocal(rvar, var)
            rstd = rowsp.tile([1, NQ], F32R, tag="rst")
            nc.scalar.sqrt(rstd, rvar)
            RM = psbcp.tile([128, 2 * NQ], F32, tag="bc")
            nc.tensor.matmul(RM[:, 0:NQ], ones_row, mu_sb,
                             start=True, stop=True)
            nc.tensor.matmul(RM[:, NQ:], ones_row, rstd,
                             start=True, stop=True)
            RMs = main.tile([128, 2 * NQ], F32, tag="rms", bufs=2)
            nc.scalar.activation(RMs, RM, AF.Identity)
            for o in range(DC):
                dcen = sqp.tile([128, NQ], BF16, tag="d", bufs=4)
                nc.gpsimd.tensor_sub(dcen, xsrc[:, o, :], RMs[:, 0:NQ])
                u = sqp.tile([128, NQ], F32, tag="u", bufs=4)
                nc.vector.tensor_mul(u, dcen, RMs[:, NQ:])
                nc.scalar.activation(xm[:, o, :], u, AF.Identity,
                                     bias=sh_col[:, o:o + 1],
                                     scale=a_col[:, o:o + 1])

        xmod = []

        def ln_chunk(ch):
            xm = xmp.tile([128, DC, NQ], BF16, name=f"xm{ch}", tag="xm", bufs=4)

            def sq_mm(s):
                for o in range(DC):
                    sq = sqp.tile([128, NQ], BF16, tag="sq", bufs=4)
                    nc.gpsimd.tensor_mul(sq, x_t[ch][:, o, :],
                                         x_t[ch][:, o, :])
                    nc.tensor.matmul(s[64:65, :], onesb_col, sq,
                                     start=(o == 0), stop=(o == DC - 1))

            modulate(x_t[ch], psln, psbc, a1, sh_msa, xm, onesb_col, sq_mm)
            xmod.append(xm)

        k_tiles = []

        def k_chunk(ch):
            kt = kqp.tile([128, DC, NQ], BF16, name=f"kT{ch}")
            for half in range(2):
                for sub in range(3):
                    mo = half * 3 + sub
                    ps = psmm.tile([128, NQ], F32, tag="mm")
                    for o in range(DC):
                        nc.tensor.matmul(
                            ps, wk[half][:, o, sub * 128:(sub + 1) * 128],
                            xmod[ch][:, o, :],
                            start=(o == 0), stop=(o == DC - 1))
                    nc.vector.tensor_copy(kt[:, mo, :], ps)
            k_tiles.append(kt)

        ln_chunk(0)
        ln_chunk(1)
        k_chunk(0)
        ln_chunk(2)
        k_chunk(1)
        ln_chunk(3)
        k_chunk(2)
        k_chunk(3)

        # xskip reload (bf16) for the out-proj residual, early dispatch
        nc.sync.dma_start(xskip, xT_r[:, :, 0:NQ])

        psbc_ctx.__exit__(None, None, None)
        psln_ctx.__exit__(None, None, None)
        xp_ctx.__exit__(None, None, None)

        # ---- Q / V / attention, pipelined.
        wq = []
        for half in range(2):
            w = wqp.tile([128, DC, 384], BF16, tag="w")
            nc.sync.dma_start(w, wqkv_r[:, :, half * 384:(half + 1) * 384])
            wq.append(w)
        wv = []
        for half in range(2):
            w = wqp.tile([128, DC, 384], BF16, tag="w")
            nc.sync.dma_start(
                w, wqkv_r[:, :, 2 * D + half * 384:2 * D + (half + 1) * 384])
            wv.append(w)
        # MLP1 weight prefetch (gpsimd queue; lands during attention)
        w1_tiles = []
        for wt in range(6):
            w = wmp.tile([128, DC, NQ], BF16, name=f"w1_{wt}", tag="wm")
            nc.gpsimd.dma_start(w, wm1_r[:, :, wt * 512:(wt + 1) * 512])
            w1_tiles.append(w)

        qT = kqp.tile([128, DC, NQ], BF16, name="qT")

        def q_unit(mo):
            half, sub = mo // 3, mo % 3
            ps = psmm.tile([128, NQ], F32, tag="mm")
            for o in range(DC):
                nc.tensor.matmul(
                    ps, wq[half][:, o, sub * 128:(sub + 1) * 128],
                    xmod[0][:, o, :],
                    start=(o == 0), stop=(o == DC - 1))
            nc.vector.tensor_copy(qT[:, mo, :], ps)

        v_tiles = []
        for tp in range(NTP):
            vt = vp.tile([128, H, DH + 1], BF16, name=f"v{tp}")
            v_tiles.append(vt)

        def v_unit(u):
            half, tp = u // NTP, u % NTP
            ch, sub = tp // 4, tp % 4
            vt = v_tiles[tp]
            if half == 0:
                nc.vector.tensor_copy(vt[:, :, DH:DH + 1],
                                      ones_f32.to_broadcast((128, H, 1)))
            ps = psmm.tile([128, NQ], F32, tag="mm")
            for o in range(DC):
                nc.tensor.matmul(
                    ps[:, 0:384],
                    xmod[ch][:, o, sub * 128:(sub + 1) * 128],
                    wv[half][:, o, :],
                    start=(o == 0), stop=(o == DC - 1))
            nc.vector.tensor_copy(
                vt[:, half * 6:(half + 1) * 6, 0:DH],
                ps[:, 0:384].rearrange("p (h d) -> p h d", h=6))

        q_unit(0)  # head 0/1 scores only need qT[:, 0, :]

        sc2_ctx = tc.tile_pool(name="sc2", bufs=2, space="PSUM")
        sc2 = sc2_ctx.__enter__()
        pso_ctx = tc.tile_pool(name="pso", bufs=2, space="PSUM")
        pso = pso_ctx.__enter__()
        atp_ctx = tc.tile_pool(name="atp", bufs=1)
        atp = atp_ctx.__enter__()

        exp_tiles = {}

        def sc_unit(h, i):
            jo, pb = h // 2, (h % 2) * DH
            ps_sc = sc2.tile([128, 2 * NQ], F32, tag="sc")
            for half in range(2):
                tp = i * 2 + half
                nc.tensor.matmul(
                    ps_sc[:, half * NQ:(half + 1) * NQ],
                    k_tiles[tp // 4][pb:pb + DH, jo,
                                     (tp % 4) * 128:(tp % 4 + 1) * 128],
                    qT[pb:pb + DH, jo, :],
                    start=True, stop=True)
            et = atp.tile([128, 2 * NQ], BF16, tag="et", bufs=16)
            nc.scalar.activation(et, ps_sc, AF.Exp, bias=0.0, scale=0.125)
            exp_tiles[(h, i)] = et

        po_tiles = {}

        def av_mm(h, tp):
            if tp == 0:
                po_tiles[h] = pso.tile([DH + 1, NQ], F32, name=f"po{h}",
                                       tag="po")
            nc.tensor.matmul(
                po_tiles[h], v_tiles[tp][:, h, :],
                exp_tiles[(h, tp // 2)][:, (tp % 2) * NQ:(tp % 2 + 1) * NQ],
                start=(tp == 0), stop=(tp == NTP - 1))

        def nrm_unit(h):
            jo, pb = h // 2, (h % 2) * DH
            po = po_tiles.pop(h)
            rrow = atp.tile([1, NQ], F32R, tag="rr", bufs=2)
            with nc.allow_low_precision(reason="f32r is 32-bit storage"):
                nc.vector.reciprocal(rrow, po[DH:DH + 1, :])
            rb = psmm.tile([128, NQ], F32, tag="mm")
            nc.tensor.matmul(rb[0:DH, :], ones_row[:, 0:DH], rrow,
                             start=True, stop=True)
            rbs = atp.tile([DH, NQ], F32, tag="rbs", bufs=2)
            nc.vector.tensor_copy(rbs, rb[0:DH, :])
            nc.vector.tensor_mul(oT[pb:pb + DH, jo, :], po[0:DH, :], rbs)
            for i in range(NTP // 2):
                del exp_tiles[(h, i)]

        # heads 0-1: scores interleaved with the Q/V fill work
        fill = [lambda mo=mo: q_unit(mo) for mo in range(1, DC)]
        fill += [lambda u=u: v_unit(u) for u in range(2 * NTP)]
        fi = 0
        for h in range(2):
            for i in range(NTP // 2):
                sc_unit(h, i)
                take = 2 if fi < 2 else 3   # 37 fill units over 16 sc units
                for _ in range(take):
                    if fi < len(fill):
                        fill[fi]()
                        fi += 1
        while fi < len(fill):
            fill[fi]()
            fi += 1
        # steady state: SC(h) with AV(h-2) interleaved
        for h in range(2, H):
            for i in range(NTP // 2):
                sc_unit(h, i)
                av_mm(h - 2, 2 * i)
                av_mm(h - 2, 2 * i + 1)
            nrm_unit(h - 2)
        for h in (H - 2, H - 1):
            for tp in range(NTP):
                av_mm(h, tp)
            nrm_unit(h)

        atp_ctx.__exit__(None, None, None)
        pso_ctx.__exit__(None, None, None)
        sc2_ctx.__exit__(None, None, None)

        # ---- out-proj + gated residual + interleaved LN2 stats
        psln2_ctx = tc.tile_pool(name="psln2", bufs=1, space="PSUM")
        psln2 = psln2_ctx.__enter__()
        psbc2_ctx = tc.tile_pool(name="psbc2", bufs=1, space="PSUM")
        psbc2 = psbc2_ctx.__enter__()

        wo = []
        for half in range(2):
            w = wqp.tile([128, DC, 384], BF16, tag="w")
            nc.sync.dma_start(w, wout_r[:, :, half * 384:(half + 1) * 384])
            wo.append(w)

        s_ln2 = psln2.tile([65, NQ], F32, tag="s")
        for mo in range(DC):
            half, sub = mo // 3, mo % 3
            ps = psmm.tile([128, NQ], F32, tag="mm")
            for o in range(DC):
                nc.tensor.matmul(
                    ps, wo[half][:, o, sub * 128:(sub + 1) * 128],
                    oT[:, o, :],
                    start=(o == 0), stop=(o == DC - 1))
            nc.vector.scalar_tensor_tensor(
                x2[:, mo, :], in0=ps, scalar=g_msa[:, mo:mo + 1],
                in1=xskip[:, mo, :], op0=OP.mult, op1=OP.add)
            nc.tensor.matmul(s_ln2[0:1, :], onesf_col, x2[:, mo, :],
                             start=(mo == 0), stop=(mo == DC - 1))
            sq = sqp.tile([128, NQ], BF16, tag="sq", bufs=4)
            nc.gpsimd.tensor_mul(sq, x2[:, mo, :], x2[:, mo, :])
            nc.tensor.matmul(s_ln2[64:65, :], onesb_col, sq,
                             start=(mo == 0), stop=(mo == DC - 1))

        # ---- LN2 rows + modulate -> xm2
        xm2 = xmp.tile([128, DC, NQ], BF16, name="xm2", tag="xm", bufs=4)
        mu_sb = rowsp.tile([1, NQ], F32R, tag="mu")
        nc.vector.tensor_copy(mu_sb, s_ln2[0:1, :])
        musq = rowsp.tile([1, NQ], F32, tag="msq", bufs=1)
        nc.gpsimd.tensor_mul(musq, mu_sb, mu_sb)
        var = rowsp.tile([1, NQ], F32, tag="var", bufs=1)
        nc.vector.scalar_tensor_tensor(var, in0=s_ln2[64:65, :], scalar=EPS,
                                       in1=musq, op0=OP.add, op1=OP.subtract)
        rvar = rowsp.tile([1, NQ], F32, tag="rv", bufs=1)
        nc.vector.reciprocal(rvar, var)
        rstd = rowsp.tile([1, NQ], F32R, tag="rst")
        nc.scalar.sqrt(rstd, rvar)
        RM = psbc2.tile([128, 2 * NQ], F32, tag="bc")
        nc.tensor.matmul(RM[:, 0:NQ], ones_row, mu_sb, start=True, stop=True)
        nc.tensor.matmul(RM[:, NQ:], ones_row, rstd, start=True, stop=True)
        RMs = main.tile([128, 2 * NQ], F32, tag="rms", bufs=2)
        nc.scalar.activation(RMs, RM, AF.Identity)
        for o in range(DC):
            dcen = sqp.tile([128, NQ], BF16, tag="d", bufs=4)
            nc.gpsimd.tensor_sub(dcen, x2[:, o, :], RMs[:, 0:NQ])
            u = sqp.tile([128, NQ], F32, tag="u", bufs=4)
            nc.vector.tensor_mul(u, dcen, RMs[:, NQ:])
            nc.scalar.activation(xm2[:, o, :], u, AF.Identity,
                                 bias=sh_mlp[:, o:o + 1],
                                 scale=a2[:, o:o + 1])

        psbc2_ctx.__exit__(None, None, None)
        psln2_ctx.__exit__(None, None, None)

        # ---- MLP1: hT = gelu_tanh(w1 @ xm2 + b1)
        hp_ctx = tc.tile_pool(name="hp", bufs=1)
        hp = hp_ctx.__enter__()
        h_tiles = [hp.tile([128, DC, NQ], BF16, name=f"hT{i}")
                   for i in range(4)]
        for wt in range(6):
            for sub in range(4):
                ho = wt * 4 + sub
                ps = psmm.tile([128, NQ], F32, tag="mm")
                for o in range(DC):
                    nc.tensor.matmul(
                        ps, w1_tiles[wt][:, o, sub * 128:(sub + 1) * 128],
                        xm2[:, o, :],
                        start=(o == 0), stop=(o == DC - 1))
                nc.scalar.activation(h_tiles[ho // 6][:, ho % 6, :], ps,
                                     AF.Gelu_apprx_tanh,
                                     bias=b1_sb[:, ho:ho + 1], scale=1.0)

        # ---- MLP2 + gated residual, chunkwise DMA out
        for wt in range(6):
            w_t = wmp.tile([128, HC, 128], BF16, name=f"w2_{wt}", tag="wm")
            nc.gpsimd.dma_start(w_t, wm2_r[:, :, wt * 128:(wt + 1) * 128])
            ps = psmm.tile([128, NQ], F32, tag="mm")
            for ko in range(HC):
                nc.tensor.matmul(
                    ps, w_t[:, ko, :],
                    h_tiles[ko // 6][:, ko % 6, :],
                    start=(ko == 0), stop=(ko == HC - 1))
            tmp = sqp.tile([128, NQ], F32, tag="u", bufs=4)
            nc.vector.tensor_scalar(tmp, ps, g_mlp[:, wt:wt + 1],
                                    gb2[:, wt:wt + 1], op0=OP.mult,
                                    op1=OP.add)
            nc.gpsimd.tensor_add(x2[:, wt, :], tmp, x2[:, wt, :])
            nc.sync.dma_start(outT_r[:, wt, :], x2[:, wt, :])

        hp_ctx.__exit__(None, None, None)


def _fix_module_for_walrus(nc):
    """Workarounds for this container's walrus build:
    (a) it rejects >1 sync-wait per instruction ("Too many sync wait
        commands") -> hoist extra waits onto NoOp carrier instructions;
    (b) it rejects custom Pool InstISA ("ISA wrong length") -> expand the
        tail EVENT_SEMAPHORE_RANGE_CLEAR into per-sem sem-sub-imm updates
        using the final values observed in earlier waits.
    """
    import bass_rust
    nid = [0]

    def carrier(engine, wait):
        nop = mybir.InstNoOp(name=f"wsplit_{nid[0]}", ins=[], outs=[])
        nid[0] += 1
        nop.engine = engine
        nop.sync_info = mybir.SyncInfo(on_wait=[wait], on_update=[])
        return nop

    for f in nc.m.functions:
        new_blocks = []
        for bb in f.blocks:
            sem_final = {}
            out = []
            for inst in bb.instructions:
                si = inst.sync_info
                if si is not None:
                    for w in si.on_wait:
                        if w.sync_type == "semaphore" and w.wait_mode == "sem-ge-imm":
                            sem_final[w.id] = max(sem_final.get(w.id, 0),
                                                  w.wait_value)
                if (type(inst).__name__ == "InstISA"
                        and getattr(inst, "op_name", "") ==
                        "EVENT_SEMAPHORE_RANGE_CLEAR"):
                    ad = inst.ant_dict
                    lo, hi = ad["range_first"], ad["range_last"]
                    waits = list(si.on_wait) if si else []
                    for w in waits:
                        out.append(carrier(inst.engine, w))
                    for sem_id in range(lo, hi + 1):
                        v = sem_final.get(sem_id, 0)
                        if v == 0:
                            continue
                        ev = mybir.InstEventSemaphore(
                            name=f"semclr_{nid[0]}", ins=[], outs=[])
                        nid[0] += 1
                        ev.engine = inst.engine
                        ev.sync_info = mybir.SyncInfo(
                            on_wait=[],
                            on_update=[mybir.SyncUpdate(
                                sync_type="semaphore", id=sem_id,
                                ant_name=f"clr{sem_id}",
                                update_mode="sem-sub-imm", update_value=v,
                                update_reg=None)])
                        out.append(ev)
                    continue
                if type(inst).__name__ == "InstISA":
                    raise RuntimeError(
                        f"unsupported InstISA {getattr(inst, 'op_name', '?')}")
                waits = list(si.on_wait) if si else []
                if len(waits) > 1:
                    for w in waits[:-1]:
                        out.append(carrier(inst.engine, w))
                    inst.sync_info = mybir.SyncInfo(
                        on_wait=waits[-1:], on_update=list(si.on_update))
                out.append(inst)
            nbb = bass_rust.BasicBlock(name=bb.name, instructions=out)
            for attr in ("IsExit", "IsLoopEntry", "IsPredicated"):
                try:
                    setattr(nbb, attr, getattr(bb, attr))
                except Exception:
                    pass
            new_blocks.append(nbb)
        f.blocks = new_blocks
    return nc


def _build_nc(gelu_mode="fused", prec="bf16"):
    nc = bass.Bass(
        "TRN2", target_bir_lowering=False, debug=False, enable_asserts=False,
        num_devices=8,
    )
    shapes = {
        "xT": ([D, S], BF16),
        "ones_row": ([1, 128], F32R),
        "onesb_col": ([128, 1], BF16),
        "onesf_col": ([128, 1], F32R),
        "ada_c": ([128, 36], F32),
        "n1_c": ([128, DC], F32),
        "n2_c": ([128, DC], F32),
        "w_qkvT": ([D, 3 * D], BF16),
        "w_outT": ([D, D], BF16),
        "w_m1T": ([D, HID], BF16),
        "b1_c": ([128, HC], F32),
        "w_m2T": ([HID, D], BF16),
        "b2_c": ([128, DC], F32),
    }
    dram = {k: nc.dram_tensor(k, shp, dt, kind="ExternalInput")
            for k, (shp, dt) in shapes.items()}
    dram["outT"] = nc.dram_tensor("outT", [D, NQ], F32R, kind="ExternalOutput")
    with tile.TileContext(nc) as tc:
        _body(tc, dram)
    return nc


def _ensure_fixed(nc):
    if not getattr(nc, "_walrus_fixed", False):
        _fix_module_for_walrus(nc)
        nc._walrus_fixed = True
    return nc


_NC_CACHE = {}


def _get_nc(gelu_mode="fused", prec="bf16"):
    key = (gelu_mode, prec)
    if key not in _NC_CACHE:
        _NC_CACHE[key] = _build_nc(gelu_mode, prec)
    return _NC_CACHE[key]


def _colpack(v, nch):
    """[nch*128] vector -> [128, nch] column-packed (col jo = v[jo*128+p])."""
    return np.ascontiguousarray(np.asarray(v, np.float32).reshape(nch, 128).T)


def make_in_maps(inputs, prec="bf16"):
    import ml_dtypes
    bf16 = ml_dtypes.bfloat16
    x = np.asarray(inputs["x"], np.float32)
    c = np.asarray(inputs["c"], np.float32)
    w_ada = np.asarray(inputs["w_ada"], np.float32)
    b_ada = np.asarray(inputs["b_ada"], np.float32)
    # AdaLN modulation vectors: tiny (2x 4608x768) matmul, replicated per the
    # sharding hint; column-packed per batch.
    ada = c @ w_ada.T + b_ada                      # (2, 4608)
    tr = lambda w: np.ascontiguousarray(np.asarray(w, np.float32).T.astype(bf16))
    base = {
        "ones_row": np.ones((1, 128), np.float32),
        "onesb_col": np.full((128, 1), 1.0 / D, bf16),
        "onesf_col": np.full((128, 1), 1.0 / D, np.float32),
        "n1_c": _colpack(inputs["norm1_w"], DC),
        "n2_c": _colpack(inputs["norm2_w"], DC),
        "w_qkvT": tr(inputs["w_qkv"]),
        "w_outT": tr(inputs["w_out"]),
        "w_m1T": tr(inputs["w_mlp1"]),
        "b1_c": _colpack(inputs["b_mlp1"], HC),
        "w_m2T": tr(inputs["w_mlp2"]),
        "b2_c": _colpack(inputs["b_mlp2"], DC),
    }
    in_maps = []
    for core in range(8):
        b, k = core // 4, core % 4
        xb = np.roll(x[b], -NQ * k, axis=0)        # my queries first
        m = dict(base)
        m["xT"] = np.ascontiguousarray(xb.T.astype(bf16))
        m["ada_c"] = _colpack(ada[b], 36)
        in_maps.append(m)
    return in_maps


def assemble_output(results):
    out = np.empty((2, S, D), np.float32)
    for core in range(8):
        b, k = core // 4, core % 4
        out[b, NQ * k:NQ * (k + 1)] = results[core]["outT"].T
    return out


def kernel(**inputs):
    nc = _ensure_fixed(_get_nc())
    in_maps = make_in_maps(inputs)
    res = run_bass_kernel_spmd(nc, in_maps, core_ids=list(range(8)))
    return assemble_output(res.results)


if __name__ == "__main__":
    _get_nc()
    print("build ok")
